# revision 21
# baseline (speedup 1.0000x reference)
"""BatchAll triplet loss (multi-module variant) on 8 Trainium2 NeuronCores.

Math: labels = [0..191, 0..191] -- every label appears exactly twice, so each
anchor i has exactly ONE valid positive j = (i+192) % 384.  The (i,j,k) cubic
triplet tensor therefore collapses to an (i,k) problem:

    loss_terms[i,k] = relu(d(i, p(i)) - d(i,k) + margin) * pm[i,k] * valid[i,k]
    out = sum(loss_terms) / (count(loss_terms > EPS) + EPS)

where valid excludes k in {i, p(i)} and pm = tile(weight, (2,2)).

With unit-normalized embeddings, d(i,k) = sqrt(relu(2 - 2*G[i,k]*rn_i*rn_k))
where G is the raw Gram matrix and rn = 1/||e||.  (The reference's distance
uses the normalized Gram's diagonal, which equals 1 up to 1e-7 rounding; the
constant 2 is within mutual fp32 noise.)

Weighting trick: with pmn = -pm, relu(dpos+m-d)*pm == max((d-(dpos+m))*pmn, 0)
and count(lw > EPS) == count((d-(dpos+m))*pmn > EPS) since EPS > 0.

Sharding: anchors i are blocked over the 8 cores (48 each).  Each core receives
the full embedding set TRANSPOSED and ROTATED so that its anchor slab lands at
local columns 0..47 and the positives at columns 192..239 -- one shared NEFF,
per-core data.  Each core emits its partial [sum, count]; the host reduces.

Hardware notes driving the structure (from NTFF traces):
- instructions carry at most ONE semaphore wait, so each op depends on at most
  one producer engine (Bacc legalizes violations with costly event-sem nops).
- engines execute in order: an op waiting on late data stalls everything
  behind it on that engine, so per-engine program order = readiness order.
- the PE is HAM-throttled cold (~2-4x); a few dummy matmuls during the DMA
  phase warm it before the real matmuls run.
- only sync/scalar (HWDGE) and gpsimd (SWDGE, ~6us completion latency) can
  initiate DMAs; big loads are split across the two HWDGE rings.
- a (1,384) one-lane DVE reciprocal costs 2.5us; computing 1/norm on a
  (128,3) layout and PE-transposing to rows costs ~0.5us total.
- the ACT Sqrt table load (1.3us) is pulled off the critical path by a dummy
  sqrt issued while DMAs are in flight.
"""

import os
import sys

for _p in ("/opt/trn_rl_repo", "/root/.axon_site/_ro/trn_rl_repo"):
    if _p not in sys.path:
        sys.path.append(_p)

# The SPMD dispatch path (bass2jax.run_bass_via_pjrt) takes jax.devices(), so
# the axon platform must stay visible.  If jax has not been initialized yet and
# JAX_PLATFORMS would hide it (e.g. "cpu"), clear the restriction.
if "jax" not in sys.modules and os.environ.get("JAX_PLATFORMS") in ("cpu",):
    del os.environ["JAX_PLATFORMS"]

import numpy as np

import concourse.bass as bass
import concourse.tile as tile
from concourse import mybir
from concourse.bacc import Bacc
from concourse.bass_utils import run_bass_kernel_spmd

F32 = mybir.dt.float32
ALU = mybir.AluOpType
ACT = mybir.ActivationFunctionType

B = 192          # batch (distinct labels)
N = 2 * B        # embeddings
D = 512          # embedding dim
NCORES = 8
S = N // NCORES  # anchors per core (48)
MARGIN = 0.1
EPS = 1e-8
N_WARMUP = 4     # dummy matmuls to bring the PE out of HAM throttle


def build_nc() -> bass.Bass:
    nc = Bacc()

    embt = nc.dram_tensor("embt", [D, N], F32, kind="ExternalInput")
    pmwn = nc.dram_tensor("pmwn", [S, N], F32, kind="ExternalInput")
    out = nc.dram_tensor("out", [1, 2], F32, kind="ExternalOutput")

    KC = D // 128   # contraction chunks for the Gram (4)
    RT = N // 128   # row-layout tiles / rn row chunks (3)

    with tile.TileContext(nc) as tc:
        with (
            tc.tile_pool(name="sb", bufs=1) as sb,
            tc.tile_pool(name="ps", bufs=1, space="PSUM") as ps,
        ):
            E = []
            for c in range(KC):
                e = sb.tile([128, N], F32, tag=f"E{c}")
                E.append(e)
            pm = sb.tile([S, N], F32, tag="pm")

            # ---- loads: effective HWDGE queue bandwidth is only ~55GB/s, so
            #      total bytes dominate -- embt only, split over both rings ----
            nc.sync.dma_start(out=E[0], in_=embt[0:128, :])
            nc.scalar.dma_start(out=E[1], in_=embt[128:256, :])
            nc.sync.dma_start(out=E[2], in_=embt[256:384, :])
            nc.scalar.dma_start(out=E[3], in_=embt[384:512, :])
            nc.gpsimd.dma_start(out=pm, in_=pmwn[:, :])   # needed late; SWDGE ok

            # ---- identity via iota on the (otherwise idle) gpsimd ----
            icol = sb.tile([128, 128], F32, tag="icol")
            nc.gpsimd.iota(icol, [[1, 128]], channel_multiplier=0,
                           allow_small_or_imprecise_dtypes=True)
            iprt = sb.tile([128, 1], F32, tag="iprt")
            nc.gpsimd.iota(iprt, [[0, 1]], channel_multiplier=1,
                           allow_small_or_imprecise_dtypes=True)
            ident = sb.tile([128, 128], F32, tag="ident")
            nc.gpsimd.tensor_scalar(ident, icol, iprt, None, op0=ALU.is_equal)

            # ---- warm-up scaffolding ----
            wtile = sb.tile([128, 256], F32, tag="wtile")
            nc.vector.memset(wtile, 1.0)
            ones_col = sb.tile([S, 1], F32, tag="ones_col")
            nc.vector.memset(ones_col, 1.0)
            ones_row = sb.tile([1, S], F32, tag="ones_row")
            nc.vector.memset(ones_row, 1.0)
            twos_col = sb.tile([S, 1], F32, tag="twos_col")
            nc.vector.memset(twos_col, 2.0)
            ones_c128 = sb.tile([128, 1], F32, tag="ones_c128")
            nc.vector.memset(ones_c128, 1.0)
            one_11 = sb.tile([1, 1], F32, tag="one_11")
            nc.vector.memset(one_11, 1.0)
            tdum = sb.tile([1, 1], F32, tag="tdum")
            nc.scalar.sqrt(tdum, wtile[0:1, 0:1])   # pull ACT sqrt table early

            wps = ps.tile([128, 256], F32, tag="wrb")  # slot shared with RB
            for _ in range(N_WARMUP):
                nc.tensor.matmul(wps, wtile[:, 0:128], wtile,
                                 start=True, stop=True)

            # ---- norms from embt: ACT squares, PE column-sums, then the
            #      (1,384) row moved to a (128,3) column layout with K=1
            #      matmuls so the reciprocal runs on 128 lanes ----
            SQ = []
            for c in range(KC):
                sq = sb.tile([128, N], F32, tag=f"SQ{c}")
                nc.scalar.square(sq, E[c])
                SQ.append(sq)
            nsr_ps = ps.tile([1, N], F32, tag="nsr")
            for c in range(KC):
                nc.tensor.matmul(nsr_ps, ones_c128, SQ[c],
                                 start=(c == 0), stop=(c == KC - 1))
            ns_s = sb.tile([1, N], F32, tag="ns_s")
            nc.vector.tensor_copy(ns_s, nsr_ps)
            nsc_ps = ps.tile([128, RT], F32, tag="nsc")
            for j in range(RT):
                nc.tensor.matmul(nsc_ps[:, j:j + 1],
                                 ns_s[:, j * 128:(j + 1) * 128], one_11,
                                 start=True, stop=True)
            nrm_col = sb.tile([128, RT], F32, tag="nrm_col")
            nc.scalar.sqrt(nrm_col, nsc_ps)
            rn_col = sb.tile([128, RT], F32, tag="rn_col")   # 1/||e||
            nc.vector.reciprocal(rn_col, nrm_col)

            # ---- Gram slab: G[a,k] = sum_d embt[d,a] * embt[d,k] ----
            g_ps = ps.tile([S, N], F32, tag="G")
            for c in range(KC):
                nc.tensor.matmul(g_ps, E[c][:, 0:S], E[c],
                                 start=(c == 0), stop=(c == KC - 1))

            # ---- -2*rn_a row scale of G (DVE work while PE transposes) ----
            rnam2 = sb.tile([S, 1], F32, tag="rnam2")        # -2 * rn[slab]
            nc.vector.tensor_scalar_mul(rnam2, rn_col[0:S, 0:1], -2.0)
            x1 = sb.tile([S, N], F32, tag="X1")              # -2 rn_a G
            nc.vector.tensor_scalar_mul(x1, g_ps, rnam2)

            # ---- rn to rows via PE transposes; copies split DVE/ACT ----
            rn_t = []
            for j in range(RT):
                rj_ps = ps.tile([1, 128], F32, tag=f"rnT{j}")
                nc.tensor.transpose(rj_ps, rn_col[:, j:j + 1], ident)
                rj = sb.tile([1, 128], F32, tag=f"rn_t{j}")
                if j == 1:
                    nc.scalar.copy(rj, rj_ps)      # gpsimd can't read PSUM
                else:
                    nc.vector.tensor_copy(rj, rj_ps)
                rn_t.append(rj)

            # ---- RB[a,k] = rn_k broadcast (rank-1, 128 cols per chunk) ----
            rb_ps = ps.tile([S, N], F32, tag="wrb")
            for j in range(RT):
                nc.tensor.matmul(rb_ps[:, j * 128:(j + 1) * 128], ones_row,
                                 rn_t[j], start=True, stop=True)

            # ---- d2 = relu(2 - 2 * G * rn_a * rn_k) ----
            t1 = sb.tile([S, N], F32, tag="T1")              # -2 rn_a rn_k G
            nc.vector.tensor_mul(t1, x1, rb_ps)
            d2 = sb.tile([S, N], F32, tag="D2")
            nc.vector.tensor_scalar(d2, t1, 2.0, 0.0, op0=ALU.add, op1=ALU.max)
            dms = sb.tile([S, N], F32, tag="dms")
            nc.scalar.sqrt(dms, d2)

            # ---- positive distance straight from t1's diagonal block
            #      (pre-relu; d2_pos ~ 2 > 0 always): dpos = sqrt(t1_pos + 2),
            #      with the +2 folded into the sqrt bias ----
            dpb = sb.tile([S, S], F32, tag="dpb")
            t1pos = sb.tile([S, 1], F32, tag="t1pos")
            nc.vector.scalar_tensor_tensor(
                dpb, t1[:, B:B + S], 1.0, ident[0:S, 0:S], op0=ALU.mult,
                op1=ALU.mult, accum_out=t1pos)
            dpos = sb.tile([S, 1], F32, tag="dpos")
            nc.scalar.activation(dpos, t1pos, ACT.Sqrt, bias=twos_col, scale=1.0)
            dpos_m = sb.tile([S, 1], F32, tag="dpos_m")
            nc.vector.tensor_scalar_add(dpos_m, dpos, MARGIN)

            # ---- weighted triplet terms via the negated-weight trick ----
            lwpre = sb.tile([S, N], F32, tag="lwpre")
            nc.vector.scalar_tensor_tensor(
                lwpre, dms, dpos_m, pm, op0=ALU.subtract, op1=ALU.mult)
            stacked = sb.tile([S, 2], F32, tag="stacked")
            lw = sb.tile([S, N], F32, tag="LW")
            nc.vector.tensor_scalar(
                lw, lwpre, 0.0, 0.0, op0=ALU.max, op1=ALU.add,
                accum_out=stacked[:, 0:1])
            c01 = sb.tile([S, N], F32, tag="C01")
            nc.vector.tensor_scalar(
                c01, lwpre, EPS, 0.0, op0=ALU.is_gt, op1=ALU.add,
                accum_out=stacked[:, 1:2])

            # ---- cross-partition reduce: out[0,:] = sum_a stacked[a,:] ----
            out_ps = ps.tile([1, 2], F32, tag="out")
            nc.tensor.matmul(out_ps, ones_col, stacked, start=True, stop=True)
            outs = sb.tile([1, 2], F32, tag="outs")
            nc.scalar.copy(outs, out_ps)
            nc.sync.dma_start(out=out[:, :], in_=outs)

    nc.finalize()
    return nc


_NC_CACHE: dict = {}


def _get_nc() -> bass.Bass:
    if "nc" not in _NC_CACHE:
        _NC_CACHE["nc"] = build_nc()
    return _NC_CACHE["nc"]


def make_in_maps(output1, output2, weight):
    o1 = np.asarray(output1, dtype=np.float32)
    o2 = np.asarray(output2, dtype=np.float32)
    w = np.asarray(weight, dtype=np.float32)

    emb = np.concatenate([o1, o2], axis=0)  # (384, 512) unnormalized
    aS = np.arange(S)

    in_maps = []
    for c in range(NCORES):
        rot = (np.arange(N) + c * S) % N                  # local -> global
        embt = np.ascontiguousarray(emb[rot].T)           # (512, 384)
        pmw = np.ascontiguousarray(w[rot[:S] % B][:, rot % B])  # (48, 384)
        pmw[aS, aS] = 0.0          # k == i
        pmw[aS, B + aS] = 0.0      # k == p(i)
        in_maps.append({"embt": embt, "pmwn": -pmw})
    return in_maps


def reduce_outputs(results):
    parts = np.stack([r["out"][0] for r in results])      # (8, 2)
    total = parts.sum(axis=0, dtype=np.float32)
    return np.asarray(
        np.float32(total[0]) / (np.float32(total[1]) + np.float32(EPS)),
        dtype=np.float32)


def kernel(output1, output2, weight):
    in_maps = make_in_maps(output1, output2, weight)
    res = run_bass_kernel_spmd(_get_nc(), in_maps, core_ids=list(range(NCORES)))
    return reduce_outputs(res.results)
